# revision 23
# baseline (speedup 1.0000x reference)
"""Masked multi-head self-attention kernel for 8 Trainium2 NeuronCores.

Full module: qkv projection -> causal softmax attention (16 heads) -> out
projection, for x[4, 2048, 1024].

Sharding: core c handles batch b = c//2 and heads h0 = (c%2)*8 .. h0+8.
QKV projection + attention are fully local to a core.  The out projection
contracts over all 16 heads' channels, so the two cores of a batch exchange
their attention outputs with pairwise AllGathers (grouped by heads
{0-3},{4,5},{6,7} per query block) and each computes half of the output
columns.  Each core returns out[b][:, half].T ([512, 2048]); the host
reassembles.

Schedule notes:
 - fp16 everywhere (same PE speed as bf16, 8x the mantissa).
 - Stage 1 runs t-outer so the first matmul waits for only 1/4 of x, and
   query-block 0's attention (which needs only the t4=0 projections) is
   interleaved INTO stage 1, hiding its exp/mask/norm work under the
   projection matmuls.  PSUM: psy(2)+psv(2)+sc0(2)+psa(2) = 8 banks.
 - For qb 1-3, score k-tiles are paired into 2-bank [128,1024] PSUM tiles
   so one exp activation covers two tiles.  PSUM: pss(2x2)+psa(2)+pso(2).
 - Scores/PV matmuls are issue-interleaved (pv of the previous head-slot
   between score pairs of the current one) so PSUM WAR waits don't idle
   the PE; the flat slot pipeline runs across qb boundaries.
 - Diagonal tiles get restricted exp/PV column ranges; causal masks are
   [128,128] affine_selects on gpsimd, off the critical path (PV of a
   head runs one slot after its scores).
 - The deferred out-projection of qb-1 is spread one output-chunk per
   slot to avoid head-of-line blocking; qb3's out-projection partially
   accumulates per gather group so only the final {6,7} gather plus a
   few matmuls are exposed in the tail.
"""

import math
import os
import sys

for _p in ("/opt/trn_rl_repo", "/root/.axon_site/_ro/trn_rl_repo"):
    if os.path.isdir(_p) and _p not in sys.path:
        sys.path.insert(0, _p)
        break

import numpy as np

import concourse.bass as bass
import concourse.mybir as mybir
import concourse.tile as tile
from concourse import bacc
from concourse.bass_utils import run_bass_kernel_spmd

B, T, C, H = 4, 2048, 1024, 16
D = 64                 # head dim
NCORES = 8
HPC = H // 2           # heads per core = 8
CPC = HPC * D          # channels per core = 512
P = 128                # partitions
QB = 512               # query block
NQB = T // QB          # 4
KC = C // P            # contraction chunks for C = 8
SCALE = 1.0 / math.sqrt(D)

F32 = mybir.dt.float32
F16 = mybir.dt.float16
EXP = mybir.ActivationFunctionType.Exp

_CACHE = {}

# gather groups: heads 0-3, 4-5, 6-7
GGRP = [(0, 4), (4, 6), (6, 8)]
GRP_OF = {}
for _gi, (_s, _e) in enumerate(GGRP):
    for _h in range(_s, _e):
        GRP_OF[_h] = (_gi, _h - _s)
# out-proj contraction chunk cc -> (gather buffer, sub-chunk)
CCMAP = [(0, 0), (0, 1), (0, 2), (0, 3), (1, 0), (1, 1), (2, 0), (2, 1)]


def build():
    nc = bacc.Bacc("TRN2", num_devices=NCORES, debug=False)

    xT = nc.dram_tensor("xT", [C, T], F16, kind="ExternalInput")
    wqk = nc.dram_tensor("wqk", [C, 2 * CPC], F16, kind="ExternalInput")
    wv = nc.dram_tensor("wv", [C, CPC], F16, kind="ExternalInput")
    bqk = nc.dram_tensor("bqk", [1, 2 * CPC], F32, kind="ExternalInput")
    wout = nc.dram_tensor("wout", [C, CPC], F16, kind="ExternalInput")
    bout = nc.dram_tensor("bout", [1, CPC], F32, kind="ExternalInput")
    outT = nc.dram_tensor("outT", [CPC, T], F16, kind="ExternalOutput")

    groups = [[0, 1], [2, 3], [4, 5], [6, 7]]

    with tile.TileContext(nc) as tc:
        with (
            tc.tile_pool(name="const", bufs=1) as constp,
            tc.tile_pool(name="ytp", bufs=1) as ytp,
            tc.tile_pool(name="vaugp", bufs=1) as vaugp,
            tc.tile_pool(name="ptp", bufs=3) as ptp,
            tc.tile_pool(name="recip", bufs=4) as recipp,
            tc.tile_pool(name="bc", bufs=3) as bcp,
            tc.tile_pool(name="atv", bufs=3) as atvp,
            tc.tile_pool(name="ps_a", bufs=2, space="PSUM") as psa,
            tc.tile_pool(name="dram", bufs=1, space="DRAM") as dramp,
        ):
            # per-partition bias layouts: bq_sb[p, n] = bqk[n*128 + p]
            bq_sb = constp.tile([P, 8], F32, tag="bq")
            nc.sync.dma_start(
                bq_sb[:].rearrange("p (o n) -> p o n", o=1),
                bqk.ap().rearrange("o (n p) -> p o n", p=P),
            )
            bo_sb = constp.tile([P, 4], F32, tag="bo")
            nc.sync.dma_start(
                bo_sb[:].rearrange("p (o n) -> p o n", o=1),
                bout.ap().rearrange("o (n p) -> p o n", p=P),
            )
            ones_f32 = constp.tile([P, P], F32, tag="ones")
            nc.vector.memset(ones_f32[:], 1.0)

            # Q^T,K^T: chunk n (0-3 Q, 4-7 K), per t4 window: [128, 512]
            yts = [
                [
                    ytp.tile([P, QB], F16, name=f"yt{n}_{t4}",
                             tag=f"yt{n}_{t4}")
                    for t4 in range(4)
                ]
                for n in range(8)
            ]
            # V natural (+ones col) per head h at [h, tt, 0:65]
            vaug_all = vaugp.tile([P, HPC * 16 * 65], F16, tag="vaug")
            vaug4 = vaug_all[:].rearrange("p (h k c) -> p h k c", h=HPC, c=65)
            nc.vector.tensor_copy(
                vaug_all[:]
                .rearrange("p (k c) -> p k c", c=65)[:, :, 64:65],
                ones_f32[:, 0:HPC * 16].rearrange("p (a b) -> p a b", b=1),
            )

            warm_in = dramp.tile([1, 4], F32, name="warm_in",
                                 tag="warm_in")
            warm_out = dramp.tile([2, 4], F32, name="warm_out",
                                  tag="warm_out")
            ag_ins_q, ag_outs_q = {}, {}
            for qb in range(NQB):
                ag_ins_q[qb] = [
                    dramp.tile(
                        [(e - s) * 64, QB], F16,
                        name=f"agin{qb}_{i}", tag=f"agin{qb}_{i}",
                    )
                    for i, (s, e) in enumerate(GGRP)
                ]
                ag_outs_q[qb] = [
                    dramp.tile(
                        [2 * (e - s) * 64, QB], F16,
                        name=f"agout{qb}_{i}", tag=f"agout{qb}_{i}",
                    )
                    for i, (s, e) in enumerate(GGRP)
                ]

            # -------- shared attention helpers (used in both stages) -----
            def s_steps(qb, h, ptreg, scpool, paired):
                """Score-matmul + exp + mask steps for (qb, h).
                Returns (list of step thunks, {kt: (col, qoff)})."""
                poff = (h % 2) * 64
                diags = [(4 * qb + j, j * P) for j in range(4)]
                offs = [(kt, 0) for kt in range(4 * qb)]
                tiles = diags + offs
                if paired:
                    units = [
                        (tiles[i], tiles[i + 1])
                        for i in range(0, len(tiles), 2)
                    ]
                    width = 2 * QB
                else:
                    units = [(t,) for t in tiles]
                    width = QB
                pts = {}
                for ui, unit in enumerate(units):
                    for half, (kt, qo) in enumerate(unit):
                        pts[kt] = (ui * width + half * QB, qo)

                def step(ui):
                    unit = units[ui]
                    sc = scpool.tile([P, width], F32, tag="sc")
                    for half, (kt, qo) in enumerate(unit):
                        nc.tensor.matmul(
                            sc[:, half * QB + qo:(half + 1) * QB],
                            yts[4 + h // 2][kt // 4][
                                poff:poff + 64, (kt % 4) * P:(kt % 4 + 1) * P],
                            yts[h // 2][qb][poff:poff + 64, qo:QB],
                            start=True, stop=True,
                        )
                    col = ui * width
                    if all(qo == 0 for _, qo in unit):
                        nc.scalar.activation(
                            ptreg[:, col:col + width], sc[:],
                            EXP, scale=SCALE,
                        )
                    else:
                        for half, (kt, qo) in enumerate(unit):
                            nc.scalar.activation(
                                ptreg[:, col + half * QB + qo:
                                      col + (half + 1) * QB],
                                sc[:, half * QB + qo:(half + 1) * QB],
                                EXP, scale=SCALE,
                            )
                    for half, (kt, qo) in enumerate(unit):
                        j = kt - 4 * qb
                        if j >= 0:
                            blk = col + half * QB + j * P
                            nc.gpsimd.affine_select(
                                out=ptreg[:, blk:blk + P],
                                in_=ptreg[:, blk:blk + P],
                                compare_op=mybir.AluOpType.is_ge,
                                fill=0.0,
                                base=0,
                                pattern=[[1, P]],
                                channel_multiplier=-1,
                            )

                return [lambda ui=ui: step(ui) for ui in range(len(units))], pts

            def pv_mms(qb, h, ptreg, pts):
                """PV matmul thunks for (qb, h): diag j0 (full, start) ->
                off-diags -> diag j1..j3 (restricted, ragged stop)."""
                order = (
                    [4 * qb]
                    + list(range(0, 4 * qb))
                    + [4 * qb + j for j in (1, 2, 3)]
                )
                pa = psa.tile([P, QB], F32, tag="pa")

                def mk(i):
                    def mm():
                        kt = order[i]
                        col, qo = pts[kt]
                        nc.tensor.matmul(
                            pa[0:65, qo:QB],
                            vaug4[:, h, kt, :],
                            ptreg[:, col + qo:col + QB],
                            start=(i == 0),
                            stop=(i == len(order) - 1),
                            skip_group_check=True,
                        )
                    return mm

                return [mk(i) for i in range(len(order))], pa

            def norm_and_send(qb, h, pa):
                gi, row = GRP_OF[h]
                # reciprocal_approx_fast misreads PSUM at partition
                # offset 64 — stage through SBUF at partition 0
                sums = recipp.tile([1, QB], F32, tag="sums")
                nc.vector.tensor_copy(sums[:], pa[64:65, :])
                recip = recipp.tile([1, QB], F32, tag="recip")
                nc.vector.reciprocal_approx_fast(recip[:], sums[:])
                bc = bcp.tile([64, QB], F32, tag="bc")
                nc.gpsimd.partition_broadcast(bc[:], recip[:])
                atv = atvp.tile([64, QB], F16, tag="atv")
                nc.vector.tensor_mul(atv[:], pa[0:64, :], bc[:])
                nc.sync.dma_start(
                    ag_ins_q[qb][gi][row * 64:(row + 1) * 64, :], atv[:]
                )
                if h == GGRP[gi][1] - 1:
                    nc.gpsimd.collective_compute(
                        "AllGather",
                        mybir.AluOpType.bypass,
                        replica_groups=groups,
                        ins=[ag_ins_q[qb][gi].opt()],
                        outs=[ag_outs_q[qb][gi].opt()],
                    )

            # state threaded between stage-1 (qb0) and stage-2 (qb1-3)
            pipe = {"prev": None}

            def run_slot(qb, h, scpool, paired, extra=None):
                """One head-slot: scores/exp/mask for (qb,h) interleaved
                with PV of the previous slot, then norm+gather for it."""
                ptreg = ptp.tile(
                    [P, 4 * NQB * QB], F16, name=f"pt{qb}_{h}", tag="ptreg",
                )
                steps, pts = s_steps(qb, h, ptreg, scpool, paired)
                pvs = []
                if pipe["prev"] is not None:
                    pqb, ph, pregion, ppts = pipe["prev"]
                    pvs, pa = pv_mms(pqb, ph, pregion, ppts)
                # prime 2 score units, then alternate 1 unit : k pv mms
                kratio = max(1, (len(pvs) + max(1, len(steps) - 2) - 1)
                             // max(1, len(steps) - 2)) if pvs else 1
                pi = vi = 0
                while pi < min(2, len(steps)):
                    steps[pi]()
                    pi += 1
                while pi < len(steps) or vi < len(pvs):
                    if pi < len(steps):
                        steps[pi]()
                        pi += 1
                    for _ in range(kratio):
                        if vi < len(pvs):
                            pvs[vi]()
                            vi += 1
                if pipe["prev"] is not None:
                    pqb, ph, _, _ = pipe["prev"]
                    norm_and_send(pqb, ph, pa)
                if extra is not None:
                    extra()
                pipe["prev"] = (qb, h, ptreg, pts)

            # ---------------- stage 1: qkv projection + qb0 attention ----
            with (
                tc.tile_pool(name="xtp", bufs=1) as xtp,
                tc.tile_pool(name="wqp", bufs=1) as wqp,
                tc.tile_pool(name="wvp", bufs=1) as wvp,
                tc.tile_pool(name="ps_y", bufs=4, space="PSUM") as psy,
                tc.tile_pool(name="ps_v", bufs=2, space="PSUM") as psv,
            ):
                xts = [
                    [
                        xtp.tile([P, QB], F16, name=f"xt{kc}_{t4}",
                                 tag=f"xt{kc}_{t4}")
                        for t4 in range(4)
                    ]
                    for kc in range(KC)
                ]
                wq_sb = [
                    wqp.tile([P, 2 * CPC], F16, name=f"wq{kc}",
                             tag=f"wq{kc}")
                    for kc in range(KC)
                ]
                wv_sb = [
                    wvp.tile([P, CPC], F16, name=f"wv{kc}", tag=f"wv{kc}")
                    for kc in range(KC)
                ]
                # DMA order: x t4=0 slices, qk weights, v weights, rest of x
                for kc in range(KC):
                    nc.sync.dma_start(
                        xts[kc][0][:], xT[kc * P:(kc + 1) * P, 0:QB]
                    )
                for kc in range(KC):
                    nc.sync.dma_start(
                        wq_sb[kc][:], wqk[kc * P:(kc + 1) * P, :]
                    )
                for kc in range(KC):
                    nc.sync.dma_start(
                        wv_sb[kc][:], wv[kc * P:(kc + 1) * P, :]
                    )
                for t4 in range(1, 4):
                    for kc in range(KC):
                        nc.sync.dma_start(
                            xts[kc][t4][:],
                            xT[kc * P:(kc + 1) * P, t4 * QB:(t4 + 1) * QB],
                        )

                def qk_chunk(n, t4):
                    py = psy.tile([P, QB], F32, tag="py")
                    for kc in range(KC):
                        nc.tensor.matmul(
                            py[:],
                            wq_sb[kc][:, n * P:(n + 1) * P],
                            xts[kc][t4][:],
                            start=(kc == 0),
                            stop=(kc == KC - 1),
                        )
                    # bias add on the (idle) scalar engine so the DVE
                    # queue drains before the stage-2 PSUM handover
                    nc.scalar.add(yts[n][t4][:], py[:], bq_sb[:, n:n + 1])

                def v_block(tt):
                    t4, j = tt // 4, tt % 4
                    pv = psv.tile([P, CPC], F32, tag="pv")
                    for kc in range(KC):
                        nc.tensor.matmul(
                            pv[:],
                            xts[kc][t4][:, j * P:(j + 1) * P],
                            wv_sb[kc][:],
                            start=(kc == 0),
                            stop=(kc == KC - 1),
                        )
                    nc.vector.tensor_copy(
                        vaug4[:, :, tt, 0:64],
                        pv[:].rearrange("p (h c) -> p h c", c=64),
                    )

                # warm up the collective stream: the first AllGather
                # pays ~11.5us of ring init; absorb it under stage 1
                nc.gpsimd.collective_compute(
                    "AllGather",
                    mybir.AluOpType.bypass,
                    replica_groups=groups,
                    ins=[warm_in.opt()],
                    outs=[warm_out.opt()],
                )
                for n in range(8):
                    qk_chunk(n, 0)
                for tt in range(4):
                    v_block(tt)
                for t4 in range(1, 4):
                    for tt in range(4 * t4, 4 * t4 + 4):
                        v_block(tt)
                    for n in range(8):
                        qk_chunk(n, t4)

            # ---------------- stage 2: qb 1-3 attention + out proj -------
            with (
                tc.tile_pool(name="w2", bufs=1) as w2p,
                tc.tile_pool(name="agr", bufs=2) as agrp,
                tc.tile_pool(name="outsb", bufs=4) as outsbp,
                tc.tile_pool(name="ps_s", bufs=2, space="PSUM") as pss,
                tc.tile_pool(name="ps_o", bufs=2, space="PSUM") as pso,
            ):
                w2sb = w2p.tile([P, KC * CPC], F16, tag="w2")
                nc.sync.dma_start(
                    w2sb[:].rearrange("p (c n) -> p c n", n=CPC),
                    wout.ap().rearrange("(c p) n -> p c n", p=P),
                )
                w23 = w2sb[:].rearrange("p (c n) -> p c n", n=CPC)

                def load_agr(qb, gi):
                    ncch = 2 * (GGRP[gi][1] - GGRP[gi][0]) * 64 // P
                    agr = agrp.tile(
                        [P, ncch * QB], F16,
                        name=f"agr{qb}_{gi}", tag=f"agr{gi}",
                    )
                    nc.sync.dma_start(
                        agr[:].rearrange("p (c n) -> p c n", n=QB),
                        ag_outs_q[qb][gi][:].rearrange("(c p) n -> p c n",
                                                       p=P),
                    )
                    return agr[:].rearrange("p (c n) -> p c n", n=QB)

                def outproj_oc(oc, agr3s, ccs, po, start, stop):
                    for idx, cc in enumerate(ccs):
                        gi, sub = CCMAP[cc]
                        nc.tensor.matmul(
                            po[:],
                            w23[:, cc, oc * P:(oc + 1) * P],
                            agr3s[gi][:, sub, :],
                            start=(start and idx == 0),
                            stop=(stop and idx == len(ccs) - 1),
                        )

                def outproj_finish(qb, oc, po, use_scalar=False):
                    osb = outsbp.tile([P, QB], F16, tag="osb")
                    if use_scalar:
                        nc.scalar.add(osb[:], po[:], bo_sb[:, oc:oc + 1])
                    else:
                        nc.vector.tensor_scalar_add(
                            osb[:], po[:], bo_sb[:, oc:oc + 1]
                        )
                    nc.sync.dma_start(
                        outT[oc * P:(oc + 1) * P, qb * QB:(qb + 1) * QB],
                        osb[:],
                    )

                # deferred out-proj of qb-1, one oc per slot call
                defer = {"qb": None, "agr3s": None}

                def op_load01(qb):
                    def f():
                        defer["qb"] = qb
                        defer["agr3s"] = [load_agr(qb, 0), load_agr(qb, 1)]
                    return f

                def op_load2(qb):
                    def f():
                        defer["agr3s"].append(load_agr(qb, 2))
                    return f

                def op_oc(oc):
                    def f():
                        po = pso.tile([P, QB], F32, tag="po")
                        outproj_oc(oc, defer["agr3s"], list(range(KC)),
                                   po, True, True)
                        outproj_finish(defer["qb"], oc, po)
                    return f

                lastq = {}

                def lastq_load01():
                    lastq["agr3s"] = {0: load_agr(0, 0), 1: load_agr(0, 1)}

                # qb0's head-slots are interleaved mid-stream so its g0/g1
                # gathers hide under qb2/qb3 attention; only the tiny {6,7}
                # gather remains at the tail, hidden under qb3's deferred
                # out-projection.
                seq = (
                    [(1, h) for h in range(HPC)]
                    + [(0, 0), (0, 1), (0, 2), (0, 3)]
                    + [(2, h) for h in range(HPC)]
                    + [(0, 4), (0, 5)]
                    + [(3, h) for h in range(HPC)]
                    + [(0, 6), (0, 7)]
                )
                extras = {
                    (0, 1): op_load01(1),
                    (0, 2): op_load2(1),
                    (2, 0): op_oc(0), (2, 1): op_oc(1),
                    (2, 2): op_oc(2), (2, 3): op_oc(3),
                    (2, 7): op_load01(2),
                    (0, 5): op_load2(2),
                    (3, 0): op_oc(0), (3, 1): op_oc(1),
                    (3, 2): op_oc(2), (3, 3): op_oc(3),
                    (3, 4): lastq_load01,
                    (3, 7): op_load01(3),
                    (0, 7): op_load2(3),
                }
                for qb, h in seq:
                    ex = extras.get((qb, h))
                    if (qb, h) == (3, 4):
                        # two extras share this slot
                        exs = [extras[(3, 4)]]
                        ex = lambda: [f() for f in exs]
                    run_slot(qb, h, pss, paired=True, extra=ex)

                # close out: pv + norm + final gather for (0, 7)
                pqb, ph, pregion, ppts = pipe["prev"]
                pvs, pa = pv_mms(pqb, ph, pregion, ppts)
                for mm in pvs:
                    mm()
                norm_and_send(pqb, ph, pa)

                # tail: qb0's cc0-5 partials FIRST (they depend only on
                # agr0/1, loaded mid-stream) — ~8us of matmuls that hide
                # BOTH final gathers (g2(qb3) and g2(qb0)); qb0's four
                # partials live in the two free score pair-tiles so pso
                # stays free for qb3's out-projection (no ring deadlock)
                agr3s = lastq["agr3s"]
                poA = pss.tile([P, 2 * QB], F32, tag="sc", name="poA")
                poB = pss.tile([P, 2 * QB], F32, tag="sc", name="poB")
                halves = [poA[:, 0:QB], poA[:, QB:2 * QB],
                          poB[:, 0:QB], poB[:, QB:2 * QB]]
                for oc in range(4):
                    outproj_oc(oc, agr3s, [0, 1, 2, 3, 4, 5],
                               halves[oc], True, False)
                agr3s[2] = load_agr(0, 2)
                # qb3's full out-projection (agr2(qb3) lands while the
                # partials above stream)
                for oc in range(4):
                    po = pso.tile([P, QB], F32, tag="po",
                                  name=f"poq3_{oc}")
                    outproj_oc(oc, defer["agr3s"], list(range(KC)),
                               po, True, True)
                    outproj_finish(3, oc, po, use_scalar=True)
                # close qb0 once g2(qb0) has landed
                for oc in range(4):
                    outproj_oc(oc, agr3s, [6, 7], halves[oc], False, True)
                    outproj_finish(0, oc, halves[oc], use_scalar=True)

    nc.compile()
    return nc


def kernel(x, w_qkv, b_qkv, w_out, b_out):
    x = np.asarray(x, dtype=np.float32)
    w_qkv = np.asarray(w_qkv, dtype=np.float32)
    b_qkv = np.asarray(b_qkv, dtype=np.float32)
    w_out = np.asarray(w_out, dtype=np.float32)
    b_out = np.asarray(b_out, dtype=np.float32)

    if "nc" not in _CACHE:
        _CACHE["nc"] = build()
    nc = _CACHE["nc"]

    # V bias passes through softmax unchanged; fold it into the out bias
    bv_all = b_qkv[2 * C:3 * C]

    in_maps = []
    for c in range(NCORES):
        b = c // 2
        h0 = (c % 2) * HPC
        cols = slice(h0 * D, h0 * D + CPC)
        wqk_c = np.concatenate(
            [w_qkv[:, cols], w_qkv[:, C:][:, cols]], axis=1
        )
        wv_c = w_qkv[:, 2 * C:][:, cols]
        bqk_c = np.concatenate(
            [b_qkv[cols], b_qkv[C:][cols]]
        ).reshape(1, 2 * CPC)
        half = slice((c % 2) * CPC, (c % 2) * CPC + CPC)
        wo = w_out[:, half]
        # rows permuted to the gathered channel order:
        # [even h0-3, odd h0-3, even h4-5, odd h4-5, even h6-7, odd h6-7]
        wo_perm = np.concatenate(
            [wo[0:256], wo[512:768],
             wo[256:384], wo[768:896],
             wo[384:512], wo[896:1024]], axis=0
        )
        bout_eff = b_out[half] + bv_all @ w_out[:, half]
        in_maps.append({
            "xT": np.ascontiguousarray(x[b].T.astype(np.float16)),
            "wqk": np.ascontiguousarray(wqk_c.astype(np.float16)),
            "wv": np.ascontiguousarray(wv_c.astype(np.float16)),
            "bqk": np.ascontiguousarray(bqk_c),
            "wout": np.ascontiguousarray(wo_perm.astype(np.float16)),
            "bout": np.ascontiguousarray(bout_eff).reshape(1, CPC),
        })

    kwargs = {}
    tdir = os.environ.get("KERNEL_TRACE_DIR")
    if tdir:
        kwargs = dict(trace=True, tmpdir=tdir)
    res = run_bass_kernel_spmd(
        nc, in_maps, core_ids=list(range(NCORES)), **kwargs
    )
    _CACHE["last_results"] = res

    out = np.empty((B, T, C), dtype=np.float32)
    for c in range(NCORES):
        b = c // 2
        half = slice((c % 2) * CPC, (c % 2) * CPC + CPC)
        out[b][:, half] = res.results[c]["outT"].T
    return out


# revision 25
# speedup vs baseline: 1.0112x; 1.0112x over previous
"""Masked multi-head self-attention kernel for 8 Trainium2 NeuronCores.

Full module: qkv projection -> causal softmax attention (16 heads) -> out
projection, for x[4, 2048, 1024].

Sharding: core c handles batch b = c//2 and heads h0 = (c%2)*8 .. h0+8.
QKV projection + attention are fully local to a core.  The out projection
contracts over all 16 heads' channels, so the two cores of a batch exchange
their attention outputs with pairwise AllGathers (grouped by heads
{0-3},{4,5},{6,7} per query block) and each computes half of the output
columns.  Each core returns out[b][:, half].T ([512, 2048]); the host
reassembles.

Schedule notes:
 - fp16 everywhere (same PE speed as bf16, 8x the mantissa).
 - Stage 1 runs t-outer so the first matmul waits for only 1/4 of x, and
   query-block 0's attention (which needs only the t4=0 projections) is
   interleaved INTO stage 1, hiding its exp/mask/norm work under the
   projection matmuls.  PSUM: psy(2)+psv(2)+sc0(2)+psa(2) = 8 banks.
 - For qb 1-3, score k-tiles are paired into 2-bank [128,1024] PSUM tiles
   so one exp activation covers two tiles.  PSUM: pss(2x2)+psa(2)+pso(2).
 - Scores/PV matmuls are issue-interleaved (pv of the previous head-slot
   between score pairs of the current one) so PSUM WAR waits don't idle
   the PE; the flat slot pipeline runs across qb boundaries.
 - Diagonal tiles get restricted exp/PV column ranges; causal masks are
   [128,128] affine_selects on gpsimd, off the critical path (PV of a
   head runs one slot after its scores).
 - The deferred out-projection of qb-1 is spread one output-chunk per
   slot to avoid head-of-line blocking; qb3's out-projection partially
   accumulates per gather group so only the final {6,7} gather plus a
   few matmuls are exposed in the tail.
"""

import math
import os
import sys

for _p in ("/opt/trn_rl_repo", "/root/.axon_site/_ro/trn_rl_repo"):
    if os.path.isdir(_p) and _p not in sys.path:
        sys.path.insert(0, _p)
        break

import numpy as np

import concourse.bass as bass
import concourse.mybir as mybir
import concourse.tile as tile
from concourse import bacc
from concourse.bass_utils import run_bass_kernel_spmd

B, T, C, H = 4, 2048, 1024, 16
D = 64                 # head dim
NCORES = 8
HPC = H // 2           # heads per core = 8
CPC = HPC * D          # channels per core = 512
P = 128                # partitions
QB = 512               # query block
NQB = T // QB          # 4
KC = C // P            # contraction chunks for C = 8
SCALE = 1.0 / math.sqrt(D)

F32 = mybir.dt.float32
F16 = mybir.dt.float16
EXP = mybir.ActivationFunctionType.Exp

_CACHE = {}

# gather groups: heads 0-3, 4-5, 6-7
GGRP = [(0, 4), (4, 6), (6, 8)]
GRP_OF = {}
for _gi, (_s, _e) in enumerate(GGRP):
    for _h in range(_s, _e):
        GRP_OF[_h] = (_gi, _h - _s)
# out-proj contraction chunk cc -> (gather buffer, sub-chunk)
CCMAP = [(0, 0), (0, 1), (0, 2), (0, 3), (1, 0), (1, 1), (2, 0), (2, 1)]


def build():
    nc = bacc.Bacc("TRN2", num_devices=NCORES, debug=False)

    xT = nc.dram_tensor("xT", [C, T], F16, kind="ExternalInput")
    wqk = nc.dram_tensor("wqk", [C, 2 * CPC], F16, kind="ExternalInput")
    wv = nc.dram_tensor("wv", [C, CPC], F16, kind="ExternalInput")
    bqk = nc.dram_tensor("bqk", [1, 2 * CPC], F32, kind="ExternalInput")
    wout = nc.dram_tensor("wout", [C, CPC], F16, kind="ExternalInput")
    bout = nc.dram_tensor("bout", [1, CPC], F32, kind="ExternalInput")
    outT = nc.dram_tensor("outT", [CPC, T], F16, kind="ExternalOutput")

    groups = [[0, 1], [2, 3], [4, 5], [6, 7]]

    with tile.TileContext(nc) as tc:
        with (
            tc.tile_pool(name="const", bufs=1) as constp,
            tc.tile_pool(name="ytp", bufs=1) as ytp,
            tc.tile_pool(name="vaugp", bufs=1) as vaugp,
            tc.tile_pool(name="ptp", bufs=4) as ptp,
            tc.tile_pool(name="recip", bufs=4) as recipp,
            tc.tile_pool(name="bc", bufs=3) as bcp,
            tc.tile_pool(name="atv", bufs=3) as atvp,
            tc.tile_pool(name="ps_a", bufs=2, space="PSUM") as psa,
            tc.tile_pool(name="dram", bufs=1, space="DRAM") as dramp,
        ):
            # per-partition bias layouts: bq_sb[p, n] = bqk[n*128 + p]
            bq_sb = constp.tile([P, 8], F32, tag="bq")
            nc.sync.dma_start(
                bq_sb[:].rearrange("p (o n) -> p o n", o=1),
                bqk.ap().rearrange("o (n p) -> p o n", p=P),
            )
            bo_sb = constp.tile([P, 4], F32, tag="bo")
            nc.sync.dma_start(
                bo_sb[:].rearrange("p (o n) -> p o n", o=1),
                bout.ap().rearrange("o (n p) -> p o n", p=P),
            )
            ones_f32 = constp.tile([P, P], F32, tag="ones")
            nc.vector.memset(ones_f32[:], 1.0)

            # Q^T,K^T: chunk n (0-3 Q, 4-7 K), per t4 window: [128, 512]
            yts = [
                [
                    ytp.tile([P, QB], F16, name=f"yt{n}_{t4}",
                             tag=f"yt{n}_{t4}")
                    for t4 in range(4)
                ]
                for n in range(8)
            ]
            # V natural (+ones col) per head h at [h, tt, 0:65]
            vaug_all = vaugp.tile([P, HPC * 16 * 65], F16, tag="vaug")
            vaug4 = vaug_all[:].rearrange("p (h k c) -> p h k c", h=HPC, c=65)
            nc.vector.tensor_copy(
                vaug_all[:]
                .rearrange("p (k c) -> p k c", c=65)[:, :, 64:65],
                ones_f32[:, 0:HPC * 16].rearrange("p (a b) -> p a b", b=1),
            )

            warm_in = dramp.tile([1, 4], F32, name="warm_in",
                                 tag="warm_in")
            warm_out = dramp.tile([2, 4], F32, name="warm_out",
                                  tag="warm_out")
            ag_ins_q, ag_outs_q = {}, {}
            for qb in range(NQB):
                ag_ins_q[qb] = [
                    dramp.tile(
                        [(e - s) * 64, QB], F16,
                        name=f"agin{qb}_{i}", tag=f"agin{qb}_{i}",
                    )
                    for i, (s, e) in enumerate(GGRP)
                ]
                ag_outs_q[qb] = [
                    dramp.tile(
                        [2 * (e - s) * 64, QB], F16,
                        name=f"agout{qb}_{i}", tag=f"agout{qb}_{i}",
                    )
                    for i, (s, e) in enumerate(GGRP)
                ]

            # -------- shared attention helpers (used in both stages) -----
            def s_steps(qb, h, ptreg, scpool, paired):
                """Score-matmul + exp + mask steps for (qb, h).
                Returns (list of step thunks, {kt: (col, qoff)})."""
                poff = (h % 2) * 64
                diags = [(4 * qb + j, j * P) for j in range(4)]
                offs = [(kt, 0) for kt in range(4 * qb)]
                tiles = diags + offs
                if paired:
                    units = [
                        (tiles[i], tiles[i + 1])
                        for i in range(0, len(tiles), 2)
                    ]
                    width = 2 * QB
                else:
                    units = [(t,) for t in tiles]
                    width = QB
                pts = {}
                for ui, unit in enumerate(units):
                    for half, (kt, qo) in enumerate(unit):
                        pts[kt] = (ui * width + half * QB, qo)

                def step(ui):
                    unit = units[ui]
                    sc = scpool.tile([P, width], F32, tag="sc")
                    for half, (kt, qo) in enumerate(unit):
                        nc.tensor.matmul(
                            sc[:, half * QB + qo:(half + 1) * QB],
                            yts[4 + h // 2][kt // 4][
                                poff:poff + 64, (kt % 4) * P:(kt % 4 + 1) * P],
                            yts[h // 2][qb][poff:poff + 64, qo:QB],
                            start=True, stop=True,
                        )
                    col = ui * width
                    if all(qo == 0 for _, qo in unit):
                        nc.scalar.activation(
                            ptreg[:, col:col + width], sc[:],
                            EXP, scale=SCALE,
                        )
                    else:
                        for half, (kt, qo) in enumerate(unit):
                            nc.scalar.activation(
                                ptreg[:, col + half * QB + qo:
                                      col + (half + 1) * QB],
                                sc[:, half * QB + qo:(half + 1) * QB],
                                EXP, scale=SCALE,
                            )
                    for half, (kt, qo) in enumerate(unit):
                        j = kt - 4 * qb
                        if j >= 0:
                            blk = col + half * QB + j * P
                            nc.gpsimd.affine_select(
                                out=ptreg[:, blk:blk + P],
                                in_=ptreg[:, blk:blk + P],
                                compare_op=mybir.AluOpType.is_ge,
                                fill=0.0,
                                base=0,
                                pattern=[[1, P]],
                                channel_multiplier=-1,
                            )

                return [lambda ui=ui: step(ui) for ui in range(len(units))], pts

            def pv_mms(qb, h, ptreg, pts):
                """PV matmul thunks for (qb, h): diag j0 (full, start) ->
                off-diags -> diag j1..j3 (restricted, ragged stop)."""
                order = (
                    [4 * qb]
                    + list(range(0, 4 * qb))
                    + [4 * qb + j for j in (1, 2, 3)]
                )
                pa = psa.tile([P, QB], F32, tag="pa")

                def mk(i):
                    def mm():
                        kt = order[i]
                        col, qo = pts[kt]
                        nc.tensor.matmul(
                            pa[0:65, qo:QB],
                            vaug4[:, h, kt, :],
                            ptreg[:, col + qo:col + QB],
                            start=(i == 0),
                            stop=(i == len(order) - 1),
                            skip_group_check=True,
                        )
                    return mm

                return [mk(i) for i in range(len(order))], pa

            def norm_and_send(qb, h, pa):
                gi, row = GRP_OF[h]
                # reciprocal_approx_fast misreads PSUM at partition
                # offset 64 — stage through SBUF at partition 0
                sums = recipp.tile([1, QB], F32, tag="sums")
                nc.vector.tensor_copy(sums[:], pa[64:65, :])
                recip = recipp.tile([1, QB], F32, tag="recip")
                nc.vector.reciprocal_approx_fast(recip[:], sums[:])
                bc = bcp.tile([64, QB], F32, tag="bc")
                nc.gpsimd.partition_broadcast(bc[:], recip[:])
                atv = atvp.tile([64, QB], F16, tag="atv")
                nc.vector.tensor_mul(atv[:], pa[0:64, :], bc[:])
                nc.sync.dma_start(
                    ag_ins_q[qb][gi][row * 64:(row + 1) * 64, :], atv[:]
                )
                if h == GGRP[gi][1] - 1:
                    nc.gpsimd.collective_compute(
                        "AllGather",
                        mybir.AluOpType.bypass,
                        replica_groups=groups,
                        ins=[ag_ins_q[qb][gi].opt()],
                        outs=[ag_outs_q[qb][gi].opt()],
                    )

            # state threaded between stage-1 (qb0) and stage-2 (qb1-3)
            pipe = {"prev": None}

            def run_slot(qb, h, scpool, paired, extra=None):
                """One head-slot: scores/exp/mask for (qb,h) interleaved
                with PV of the previous slot, then norm+gather for it."""
                ptreg = ptp.tile(
                    [P, 4 * NQB * QB], F16, name=f"pt{qb}_{h}", tag="ptreg",
                )
                steps, pts = s_steps(qb, h, ptreg, scpool, paired)
                pvs = []
                if pipe["prev"] is not None:
                    pqb, ph, pregion, ppts = pipe["prev"]
                    pvs, pa = pv_mms(pqb, ph, pregion, ppts)
                # prime 3 score units, then alternate 1 unit : k pv mms
                kratio = max(1, (len(pvs) + max(1, len(steps) - 3) - 1)
                             // max(1, len(steps) - 3)) if pvs else 1
                pi = vi = 0
                while pi < min(3, len(steps)):
                    steps[pi]()
                    pi += 1
                while pi < len(steps) or vi < len(pvs):
                    if pi < len(steps):
                        steps[pi]()
                        pi += 1
                    for _ in range(kratio):
                        if vi < len(pvs):
                            pvs[vi]()
                            vi += 1
                if pipe["prev"] is not None:
                    pqb, ph, _, _ = pipe["prev"]
                    norm_and_send(pqb, ph, pa)
                if extra is not None:
                    extra()
                pipe["prev"] = (qb, h, ptreg, pts)

            # ---------------- stage 1: qkv projection + qb0 attention ----
            with (
                tc.tile_pool(name="xtp", bufs=1) as xtp,
                tc.tile_pool(name="wqp", bufs=1) as wqp,
                tc.tile_pool(name="wvp", bufs=1) as wvp,
                tc.tile_pool(name="ps_y", bufs=4, space="PSUM") as psy,
                tc.tile_pool(name="ps_v", bufs=2, space="PSUM") as psv,
            ):
                xts = [
                    [
                        xtp.tile([P, QB], F16, name=f"xt{kc}_{t4}",
                                 tag=f"xt{kc}_{t4}")
                        for t4 in range(4)
                    ]
                    for kc in range(KC)
                ]
                wq_sb = [
                    wqp.tile([P, 2 * CPC], F16, name=f"wq{kc}",
                             tag=f"wq{kc}")
                    for kc in range(KC)
                ]
                wv_sb = [
                    wvp.tile([P, CPC], F16, name=f"wv{kc}", tag=f"wv{kc}")
                    for kc in range(KC)
                ]
                # DMA order: x t4=0 slices, qk weights, v weights, rest of x
                for kc in range(KC):
                    nc.sync.dma_start(
                        xts[kc][0][:], xT[kc * P:(kc + 1) * P, 0:QB]
                    )
                for kc in range(KC):
                    nc.sync.dma_start(
                        wq_sb[kc][:], wqk[kc * P:(kc + 1) * P, :]
                    )
                for kc in range(KC):
                    nc.sync.dma_start(
                        wv_sb[kc][:], wv[kc * P:(kc + 1) * P, :]
                    )
                for t4 in range(1, 4):
                    for kc in range(KC):
                        nc.sync.dma_start(
                            xts[kc][t4][:],
                            xT[kc * P:(kc + 1) * P, t4 * QB:(t4 + 1) * QB],
                        )

                def qk_chunk(n, t4):
                    py = psy.tile([P, QB], F32, tag="py")
                    for kc in range(KC):
                        nc.tensor.matmul(
                            py[:],
                            wq_sb[kc][:, n * P:(n + 1) * P],
                            xts[kc][t4][:],
                            start=(kc == 0),
                            stop=(kc == KC - 1),
                        )
                    # bias add on the (idle) scalar engine so the DVE
                    # queue drains before the stage-2 PSUM handover
                    nc.scalar.add(yts[n][t4][:], py[:], bq_sb[:, n:n + 1])

                def v_block(tt):
                    t4, j = tt // 4, tt % 4
                    pv = psv.tile([P, CPC], F32, tag="pv")
                    for kc in range(KC):
                        nc.tensor.matmul(
                            pv[:],
                            xts[kc][t4][:, j * P:(j + 1) * P],
                            wv_sb[kc][:],
                            start=(kc == 0),
                            stop=(kc == KC - 1),
                        )
                    nc.vector.tensor_copy(
                        vaug4[:, :, tt, 0:64],
                        pv[:].rearrange("p (h c) -> p h c", c=64),
                    )

                # warm up the collective stream: the first AllGather
                # pays ~11.5us of ring init; absorb it under stage 1
                nc.gpsimd.collective_compute(
                    "AllGather",
                    mybir.AluOpType.bypass,
                    replica_groups=groups,
                    ins=[warm_in.opt()],
                    outs=[warm_out.opt()],
                )
                for n in range(8):
                    qk_chunk(n, 0)
                for tt in range(4):
                    v_block(tt)
                for t4 in range(1, 4):
                    for tt in range(4 * t4, 4 * t4 + 4):
                        v_block(tt)
                    for n in range(8):
                        qk_chunk(n, t4)

            # ---------------- stage 2: qb 1-3 attention + out proj -------
            with (
                tc.tile_pool(name="w2", bufs=1) as w2p,
                tc.tile_pool(name="agr", bufs=2) as agrp,
                tc.tile_pool(name="outsb", bufs=4) as outsbp,
                tc.tile_pool(name="ps_s", bufs=2, space="PSUM") as pss,
                tc.tile_pool(name="ps_o", bufs=2, space="PSUM") as pso,
            ):
                w2sb = w2p.tile([P, KC * CPC], F16, tag="w2")
                nc.sync.dma_start(
                    w2sb[:].rearrange("p (c n) -> p c n", n=CPC),
                    wout.ap().rearrange("(c p) n -> p c n", p=P),
                )
                w23 = w2sb[:].rearrange("p (c n) -> p c n", n=CPC)

                def load_agr(qb, gi):
                    ncch = 2 * (GGRP[gi][1] - GGRP[gi][0]) * 64 // P
                    agr = agrp.tile(
                        [P, ncch * QB], F16,
                        name=f"agr{qb}_{gi}", tag=f"agr{gi}",
                    )
                    nc.sync.dma_start(
                        agr[:].rearrange("p (c n) -> p c n", n=QB),
                        ag_outs_q[qb][gi][:].rearrange("(c p) n -> p c n",
                                                       p=P),
                    )
                    return agr[:].rearrange("p (c n) -> p c n", n=QB)

                def outproj_oc(oc, agr3s, ccs, po, start, stop):
                    for idx, cc in enumerate(ccs):
                        gi, sub = CCMAP[cc]
                        nc.tensor.matmul(
                            po[:],
                            w23[:, cc, oc * P:(oc + 1) * P],
                            agr3s[gi][:, sub, :],
                            start=(start and idx == 0),
                            stop=(stop and idx == len(ccs) - 1),
                        )

                def outproj_finish(qb, oc, po, use_scalar=False):
                    osb = outsbp.tile([P, QB], F16, tag="osb")
                    if use_scalar:
                        nc.scalar.add(osb[:], po[:], bo_sb[:, oc:oc + 1])
                    else:
                        nc.vector.tensor_scalar_add(
                            osb[:], po[:], bo_sb[:, oc:oc + 1]
                        )
                    nc.sync.dma_start(
                        outT[oc * P:(oc + 1) * P, qb * QB:(qb + 1) * QB],
                        osb[:],
                    )

                # deferred out-proj of qb-1, one oc per slot call
                defer = {"qb": None, "agr3s": None}

                def op_load01(qb):
                    def f():
                        defer["qb"] = qb
                        defer["agr3s"] = [load_agr(qb, 0), load_agr(qb, 1)]
                    return f

                def op_load2(qb):
                    def f():
                        defer["agr3s"].append(load_agr(qb, 2))
                    return f

                def op_oc(oc):
                    def f():
                        po = pso.tile([P, QB], F32, tag="po")
                        outproj_oc(oc, defer["agr3s"], list(range(KC)),
                                   po, True, True)
                        outproj_finish(defer["qb"], oc, po)
                    return f

                lastq = {}

                def lastq_load01():
                    lastq["agr3s"] = {0: load_agr(0, 0), 1: load_agr(0, 1)}

                # qb0's head-slots are interleaved mid-stream so its g0/g1
                # gathers hide under qb2/qb3 attention; only the tiny {6,7}
                # gather remains at the tail, hidden under qb3's deferred
                # out-projection.
                seq = (
                    [(1, h) for h in range(HPC)]
                    + [(0, 0), (0, 1), (0, 2), (0, 3)]
                    + [(2, h) for h in range(HPC)]
                    + [(0, 4), (0, 5)]
                    + [(3, h) for h in range(HPC)]
                    + [(0, 6), (0, 7)]
                )
                extras = {
                    (0, 1): op_load01(1),
                    (0, 2): op_load2(1),
                    (2, 0): op_oc(0), (2, 1): op_oc(1),
                    (2, 2): op_oc(2), (2, 3): op_oc(3),
                    (2, 7): op_load01(2),
                    (0, 5): op_load2(2),
                    (3, 0): op_oc(0), (3, 1): op_oc(1),
                    (3, 2): op_oc(2), (3, 3): op_oc(3),
                    (3, 4): lastq_load01,
                    (3, 7): op_load01(3),
                    (0, 7): op_load2(3),
                }
                for qb, h in seq:
                    ex = extras.get((qb, h))
                    if (qb, h) == (3, 4):
                        # two extras share this slot
                        exs = [extras[(3, 4)]]
                        ex = lambda: [f() for f in exs]
                    run_slot(qb, h, pss, paired=True, extra=ex)

                # close out: pv + norm + final gather for (0, 7)
                pqb, ph, pregion, ppts = pipe["prev"]
                pvs, pa = pv_mms(pqb, ph, pregion, ppts)
                for mm in pvs:
                    mm()
                norm_and_send(pqb, ph, pa)

                # tail part 1: qb3's out-projection (needs only g2(qb3),
                # gathered two slots ago) hides the final qb0 gather
                for oc in range(4):
                    po = pso.tile([P, QB], F32, tag="po")
                    outproj_oc(oc, defer["agr3s"], list(range(KC)),
                               po, True, True)
                    outproj_finish(3, oc, po, use_scalar=True)

                # tail part 2: qb0's out-projection, cc0-5 first (agr0/1
                # loaded mid-stream), then cc6-7 once g2(qb0) lands
                agr3s = lastq["agr3s"]
                po01 = []
                for oc in (0, 1):
                    po = pso.tile([P, QB], F32, tag="po")
                    outproj_oc(oc, agr3s, [0, 1, 2, 3, 4, 5],
                               po, True, False)
                    po01.append(po)
                po23 = pss.tile([P, 2 * QB], F32, tag="sc")
                for i, oc in enumerate((2, 3)):
                    outproj_oc(oc, agr3s, [0, 1, 2, 3, 4, 5],
                               po23[:, i * QB:(i + 1) * QB], True, False)
                agr3s[2] = load_agr(0, 2)
                for oc in (0, 1):
                    outproj_oc(oc, agr3s, [6, 7], po01[oc], False, True)
                    outproj_finish(0, oc, po01[oc], use_scalar=True)
                for i, oc in enumerate((2, 3)):
                    outproj_oc(oc, agr3s, [6, 7],
                               po23[:, i * QB:(i + 1) * QB], False, True)
                    outproj_finish(0, oc, po23[:, i * QB:(i + 1) * QB],
                                   use_scalar=True)

    nc.compile()
    return nc


def kernel(x, w_qkv, b_qkv, w_out, b_out):
    x = np.asarray(x, dtype=np.float32)
    w_qkv = np.asarray(w_qkv, dtype=np.float32)
    b_qkv = np.asarray(b_qkv, dtype=np.float32)
    w_out = np.asarray(w_out, dtype=np.float32)
    b_out = np.asarray(b_out, dtype=np.float32)

    if "nc" not in _CACHE:
        _CACHE["nc"] = build()
    nc = _CACHE["nc"]

    # V bias passes through softmax unchanged; fold it into the out bias
    bv_all = b_qkv[2 * C:3 * C]

    in_maps = []
    for c in range(NCORES):
        b = c // 2
        h0 = (c % 2) * HPC
        cols = slice(h0 * D, h0 * D + CPC)
        wqk_c = np.concatenate(
            [w_qkv[:, cols], w_qkv[:, C:][:, cols]], axis=1
        )
        wv_c = w_qkv[:, 2 * C:][:, cols]
        bqk_c = np.concatenate(
            [b_qkv[cols], b_qkv[C:][cols]]
        ).reshape(1, 2 * CPC)
        half = slice((c % 2) * CPC, (c % 2) * CPC + CPC)
        wo = w_out[:, half]
        # rows permuted to the gathered channel order:
        # [even h0-3, odd h0-3, even h4-5, odd h4-5, even h6-7, odd h6-7]
        wo_perm = np.concatenate(
            [wo[0:256], wo[512:768],
             wo[256:384], wo[768:896],
             wo[384:512], wo[896:1024]], axis=0
        )
        bout_eff = b_out[half] + bv_all @ w_out[:, half]
        in_maps.append({
            "xT": np.ascontiguousarray(x[b].T.astype(np.float16)),
            "wqk": np.ascontiguousarray(wqk_c.astype(np.float16)),
            "wv": np.ascontiguousarray(wv_c.astype(np.float16)),
            "bqk": np.ascontiguousarray(bqk_c),
            "wout": np.ascontiguousarray(wo_perm.astype(np.float16)),
            "bout": np.ascontiguousarray(bout_eff).reshape(1, CPC),
        })

    kwargs = {}
    tdir = os.environ.get("KERNEL_TRACE_DIR")
    if tdir:
        kwargs = dict(trace=True, tmpdir=tdir)
    res = run_bass_kernel_spmd(
        nc, in_maps, core_ids=list(range(NCORES)), **kwargs
    )
    _CACHE["last_results"] = res

    out = np.empty((B, T, C), dtype=np.float32)
    for c in range(NCORES):
        b = c // 2
        half = slice((c % 2) * CPC, (c % 2) * CPC + CPC)
        out[b][:, half] = res.results[c]["outT"].T
    return out
